# revision 1
# baseline (speedup 1.0000x reference)
"""Multi-head attention (softmax+1) for TRN2, 8 NeuronCores.

Sharding: data-parallel over batch B=2 (4 cores per batch) x tensor-parallel
over the 16 heads (4 heads per core).  Each core computes its 4 heads'
QKV projections, attention, and a partial output projection; the host sums
the 4 partials per batch and adds the output bias.

Per-core kernel (S=2048, DM=1024, HD=64, Hloc=4):
  QT[d,q] / KT[d,k] head-transposed layouts from x^T inputs (PE matmuls),
  V'[k, 4*65] natural layout with a ones column per head (denominator trick),
  scores^T[k,q] -> exp on ACT (scale folded into Wq) -> U^T = V'^T @ expT
  (row 64 of each head's block = softmax denominator), normalization via
  1/(1+den) broadcast (GPSIMD partition_broadcast), partial out-projection.

All matmuls run in float16 (1 cycle/row on the PE).  Matmuls are emitted in
concurrent row-group pairs wherever possible (head-pair scores on partitions
0:64/64:128; projections as half-contraction pairs alternating row groups),
which hides LDWEIGHTS and doubles array occupancy.  The attention phase is
ACT(exp)-bound and software-pipelined one chunk ahead (scores/exp lead the
V-accumulation) so the scalar engine never starves across quarter
boundaries; V-projection / out-projection / dummy matmuls fill the PE to
keep the HAM clock-gate at 8/8.
"""

import sys

if "/opt/trn_rl_repo" not in sys.path:
    sys.path.insert(0, "/opt/trn_rl_repo")

import numpy as np

import concourse.bass as bass
import concourse.mybir as mybir
import concourse.tile as tile
from concourse import bacc
from concourse.bass_utils import run_bass_kernel_spmd

F32 = mybir.dt.float32
F16 = mybir.dt.float16
EXP = mybir.ActivationFunctionType.Exp

B, S, DM = 2, 2048, 1024
H, HD = 16, 64
SCALE = HD ** -0.5
HLOC = 4              # heads per core
CD = HLOC * HD        # 256 local head dims
VW = HD + 1           # 65: V columns + ones column per head
MC = DM // 128        # 8 contraction chunks for projections
KT16 = S // 128       # 16 sequence tiles
W260 = HLOC * VW      # 260

_CACHE = {}
LAST_RESULT = None


def _build():
    nc = bacc.Bacc()
    dp = nc.declare_dram_parameter
    xq_d = dp("xq", [DM, S], F16, isOutput=False)    # query[b]^T
    xk_d = dp("xk", [DM, S], F16, isOutput=False)
    xv_d = dp("xv", [DM, S], F16, isOutput=False)
    wq_d = dp("wq", [DM, CD], F16, isOutput=False)   # (SCALE * Wq_shard)^T
    wk_d = dp("wk", [DM, CD], F16, isOutput=False)   # Wk_shard^T
    wv_d = dp("wv", [DM, W260], F16, isOutput=False)  # Wv^T 260-layout, zeros in ones-cols
    wo_d = dp("wo", [CD, DM], F16, isOutput=False)   # Wo_shard^T
    bq_d = dp("bq", [128, 2], F32, isOutput=False)   # bias cols per 128-pair (SCALE-folded)
    bk_d = dp("bk", [128, 2], F32, isOutput=False)
    bv_d = dp("bv", [1, W260], F16, isOutput=False)  # [bv_h | 1.0] blocks
    on_d = dp("ones1", [1, 128], F16, isOutput=False)
    out_d = dp("out", [S, DM], F32, isOutput=True)   # partial (pre-bo) projection

    with tile.TileContext(nc) as tc:
        with tc.tile_pool(name="weights", bufs=1) as wpool, \
             tc.tile_pool(name="persist", bufs=1) as perst:
            wq_sb = wpool.tile([128, MC, CD], F16)
            wk_sb = wpool.tile([128, MC, CD], F16)
            wv_sb = wpool.tile([128, MC, W260], F16)
            wo_sb = wpool.tile([128, 2, DM], F16)
            bq_sb = wpool.tile([128, 2], F32)
            bk_sb = wpool.tile([128, 2], F32)
            bv_sb = wpool.tile([1, W260], F16)
            on_sb = wpool.tile([1, 128], F16)

            qt_sb = perst.tile([128, 2, S], F16)   # [d(2 heads), pair, q]
            kt_sb = perst.tile([128, 2, S], F16)
            v_sb = perst.tile([128, KT16, W260], F16)  # [k, ktile, 4*(V|1)]
            at_sb = perst.tile([128, 2, S], F16)   # normalized attn out^T
            xv_sb = perst.tile([128, MC, S], F16)  # resident value^T chunks

            # ------------- Phase 1: Q and K projections ----------------
            # Half-contraction matmul pairs on alternating row groups: the
            # second matmul's LDWEIGHTS overlaps the first's stream.
            with tc.tile_pool(name="xs", bufs=16) as xs, \
                 tc.tile_pool(name="pproj", bufs=8, space="PSUM") as pproj:
                nc.sync.dma_start(out=wq_sb[:, 0, :], in_=wq_d.ap()[0:128, :])
                for src_d, w_sb, b_sb, dst in (
                    (xq_d, wq_sb, bq_sb, qt_sb),
                    (xk_d, wk_sb, bk_sb, kt_sb),
                ):
                    first_proj = dst is qt_sb
                    pss = [pproj.tile([128, 512], F32, tag="ps", name=f"ps{k}")
                           for k in range(8)]
                    xts = []
                    for m in range(MC):
                        xt = xs.tile([128, S], F16, tag="xs", name=f"xt{m}")
                        nc.sync.dma_start(out=xt[:], in_=src_d.ap()[m * 128:(m + 1) * 128, :])
                        xts.append(xt)
                        if first_proj and m + 1 < MC:
                            nc.sync.dma_start(out=wq_sb[:, m + 1, :],
                                              in_=wq_d.ap()[(m + 1) * 128:(m + 2) * 128, :])
                    for m in range(MC):
                        xt = xts[m]
                        if first_proj and m == 0:
                            nc.sync.dma_start(out=bq_sb[:], in_=bq_d.ap())
                        st, sp = (m == 0), (m == MC - 1)
                        for p in range(2):
                            for j in range(4):
                                nc.tensor.matmul(
                                    pss[p * 4 + j][:],
                                    w_sb[:, m, p * 128:(p + 1) * 128],
                                    xt[:, j * 512:(j + 1) * 512],
                                    start=st, stop=sp,
                                )
                        if first_proj and m < 2:
                            for mm in range(m * 4, m * 4 + 4):
                                nc.sync.dma_start(out=wk_sb[:, mm, :],
                                                  in_=wk_d.ap()[mm * 128:(mm + 1) * 128, :])
                            if m == 0:
                                nc.sync.dma_start(out=bk_sb[:], in_=bk_d.ap())
                    for p in range(2):
                        for j in range(4):
                            nc.vector.tensor_scalar_add(
                                dst[:, p, j * 512:(j + 1) * 512],
                                pss[p * 4 + j][:], b_sb[:, p:p + 1],
                            )
                # stage V weights/input + wo for the attention phase
                for m in range(MC):
                    nc.sync.dma_start(out=wv_sb[:, m, :], in_=wv_d.ap()[m * 128:(m + 1) * 128, :])
                nc.sync.dma_start(out=bv_sb[:], in_=bv_d.ap())
                nc.sync.dma_start(out=on_sb[:], in_=on_d.ap())
                for m in range(MC):
                    nc.sync.dma_start(out=xv_sb[:, m, :], in_=xv_d.ap()[m * 128:(m + 1) * 128, :])
                for cc in range(2):
                    nc.sync.dma_start(out=wo_sb[:, cc, :], in_=wo_d.ap()[cc * 128:(cc + 1) * 128, :])

            # ------------- Phase 2: attention, software-pipelined -----------
            with tc.tile_pool(name="psc", bufs=2, space="PSUM") as psc, \
                 tc.tile_pool(name="put", bufs=2, space="PSUM") as put, \
                 tc.tile_pool(name="expp", bufs=4) as expp, \
                 tc.tile_pool(name="obuf", bufs=3) as obuf, \
                 tc.tile_pool(name="npool", bufs=3) as npool:

                pout = None
                pv_ctx = tc.tile_pool(name="pv", bufs=2, space="PSUM")
                pv = pv_ctx.__enter__()

                def vproj_pair(k0):
                    """V projection for k-tiles k0 and k0+1."""
                    for kt in (k0, k0 + 1):
                        vps = pv.tile([128, W260], F32, tag="vps", name="vps")
                        nc.tensor.matmul(vps[:], on_sb[:], bv_sb[:], start=True, stop=False)
                        for m in range(MC):
                            nc.tensor.matmul(
                                vps[:],
                                xv_sb[:, m, kt * 128:(kt + 1) * 128],
                                wv_sb[:, m, :],
                                start=False, stop=(m == MC - 1),
                            )
                        nc.vector.tensor_copy(v_sb[:, kt, :], vps[:])

                def outproj_t(t, act_copy=False):
                    ob = obuf.tile([128, DM], F32, tag="ob", name="ob")
                    ops = [pout.tile([128, 512], F32, tag="op", name=f"op{n}")
                           for n in range(2)]
                    for cc in range(2):
                        for n in range(2):
                            nc.tensor.matmul(
                                ops[n][:],
                                at_sb[:, cc, t * 128:(t + 1) * 128],
                                wo_sb[:, cc, n * 512:(n + 1) * 512],
                                start=(cc == 0), stop=(cc == 1),
                            )
                    nc.vector.tensor_copy(ob[:, 0:512], ops[0][:])
                    if act_copy:
                        nc.scalar.copy(ob[:, 512:1024], ops[1][:])
                    else:
                        nc.vector.tensor_copy(ob[:, 512:1024], ops[1][:])
                    nc.sync.dma_start(
                        out=out_d.ap()[t * 128:(t + 1) * 128, :], in_=ob[:],
                    )

                def dummy_mm():
                    wps = pout.tile([128, 512], F32, tag="op", name="warm")
                    nc.tensor.matmul(wps[:], wo_sb[:, 0, 0:128], wo_sb[:, 0, 0:512],
                                     start=True, stop=True)

                def normalize(uts, p, q0):
                    dens, us = [], []
                    for hh in range(2):
                        den1 = npool.tile([1, 512], F32, tag="den", name=f"den{hh}")
                        nc.vector.tensor_scalar_add(den1[:], uts[hh][64:65, :], 1.0)
                        u = npool.tile([64, 512], F32, tag="u", name=f"u{hh}")
                        nc.vector.tensor_copy(u[:], uts[hh][0:64, :])
                        dens.append(den1)
                        us.append(u)
                    for hh in range(2):
                        po = 64 * hh
                        r = npool.tile([1, 512], F32, tag="r")
                        nc.vector.reciprocal_approx_fast(r[:], dens[hh][:])
                        rb = npool.tile([64, 512], F32, tag="rb")
                        nc.gpsimd.partition_broadcast(rb[:], r[:])
                        nc.vector.tensor_mul(
                            at_sb[po:po + 64, p, q0:q0 + 512], us[hh][:], rb[:])

                sched = [(p, qq, i) for p in range(2) for qq in range(4)
                         for i in range(KT16)]
                quarters = {}
                prev = None
                for g in range(len(sched) + 1):
                    if g < len(sched):
                        p, qq, i = sched[g]
                        if i == 0:
                            quarters[(p, qq)] = (
                                put.tile([65, 512], F32, tag="ut", name="ut0"),
                                put.tile([65, 512], F32, tag="ut", name="ut1"),
                            )
                        q0 = qq * 512
                        sc = psc.tile([128, 1024], F32, tag="sc")
                        for hh in range(2):
                            nc.tensor.matmul(
                                sc[:, hh * 512:(hh + 1) * 512],
                                kt_sb[64 * hh:64 * hh + 64, p, i * 128:(i + 1) * 128],
                                qt_sb[64 * hh:64 * hh + 64, p, q0:q0 + 512],
                                start=True, stop=True,
                            )
                        ex = expp.tile([128, 1024], F16, tag="ex")
                        nc.scalar.activation(out=ex[:], in_=sc[:], func=EXP)
                        if g == 0:
                            vproj_pair(0)   # k-tiles 0,1 behind the first exp
                        cur = (p, qq, i, ex)
                    else:
                        cur = None
                    if prev is not None:
                        pp, pqq, pi, pex = prev
                        fq = pp == 0 and pqq == 0
                        if fq and pi % 2 == 1 and pi < KT16 - 2:
                            vproj_pair(pi + 1)  # stays ahead of the V-MMs
                        elif not fq:
                            ot = (pqq - 1) * 4 + (pi - 8) // 2 \
                                if pp == 1 and pqq > 0 and pi >= 8 and pi % 2 == 0 \
                                else None
                            if ot is not None:
                                outproj_t(ot)
                            elif pout is not None and pi % 4 == 0:
                                dummy_mm()
                        uts = quarters[(pp, pqq)]
                        for hh in range(2):
                            h = 2 * pp + hh
                            nc.tensor.matmul(
                                uts[hh][:],
                                v_sb[:, pi, h * VW:(h + 1) * VW],
                                pex[:, hh * 512:(hh + 1) * 512],
                                start=(pi == 0), stop=(pi == KT16 - 1),
                            )
                        if pi == KT16 - 1:
                            if fq:
                                pv_ctx.__exit__(None, None, None)
                                pout_ctx = tc.tile_pool(name="pout", bufs=2,
                                                        space="PSUM")
                                pout = pout_ctx.__enter__()
                            normalize(uts, pp, pqq * 512)
                            del quarters[(pp, pqq)]
                    prev = cur
                # final q-quarter's out-projection (ACT is idle by now)
                for t in range(12, 16):
                    outproj_t(t, act_copy=True)
                pout_ctx.__exit__(None, None, None)

    nc.finalize()
    return nc


def kernel(query, key, value, Wq, bq, Wk, bk, Wv, bv, Wo, bo):
    global LAST_RESULT
    if "nc" not in _CACHE:
        _CACHE["nc"] = _build()
    nc = _CACHE["nc"]

    query = np.asarray(query, np.float32)
    key = np.asarray(key, np.float32)
    value = np.asarray(value, np.float32)
    Wq = np.asarray(Wq, np.float32)
    Wk = np.asarray(Wk, np.float32)
    Wv = np.asarray(Wv, np.float32)
    Wo = np.asarray(Wo, np.float32)
    bq = np.asarray(bq, np.float32)
    bk = np.asarray(bk, np.float32)
    bv = np.asarray(bv, np.float32)
    bo = np.asarray(bo, np.float32)

    xqT = [np.ascontiguousarray(query[b].T).astype(np.float16) for b in range(B)]
    xkT = [np.ascontiguousarray(key[b].T).astype(np.float16) for b in range(B)]
    xvT = [np.ascontiguousarray(value[b].T).astype(np.float16) for b in range(B)]

    ones1 = np.ones((1, 128), np.float16)
    in_maps = []
    for c in range(8):
        b, hg = c // 4, c % 4
        r0 = hg * CD
        wq_s = np.ascontiguousarray((Wq[r0:r0 + CD, :] * SCALE).T).astype(np.float16)
        wk_s = np.ascontiguousarray(Wk[r0:r0 + CD, :].T).astype(np.float16)
        wo_s = np.ascontiguousarray(Wo[:, r0:r0 + CD].T).astype(np.float16)
        bq_s = np.ascontiguousarray((bq[r0:r0 + CD] * SCALE).reshape(2, 128).T)  # [128,2]
        bk_s = np.ascontiguousarray(bk[r0:r0 + CD].reshape(2, 128).T)
        # V weights/bias in 260-layout: [64 cols of head | bias-1 col] x4
        wv260 = np.zeros((DM, W260), np.float32)
        bv260 = np.zeros((1, W260), np.float32)
        for hh in range(HLOC):
            wv260[:, hh * VW:hh * VW + HD] = Wv[r0 + hh * HD:r0 + (hh + 1) * HD, :].T
            bv260[0, hh * VW:hh * VW + HD] = bv[r0 + hh * HD:r0 + (hh + 1) * HD]
            bv260[0, hh * VW + HD] = 1.0
        in_maps.append({
            "xq": xqT[b], "xk": xkT[b], "xv": xvT[b],
            "wq": wq_s, "wk": wk_s, "wv": np.ascontiguousarray(wv260).astype(np.float16),
            "wo": wo_s, "bq": bq_s, "bk": bk_s, "bv": bv260.astype(np.float16),
            "ones1": ones1,
        })

    res = run_bass_kernel_spmd(nc, in_maps, core_ids=list(range(8)))
    LAST_RESULT = res

    out = np.empty((B, S, DM), np.float32)
    for b in range(B):
        acc = np.zeros((S, DM), np.float64)
        for hg in range(4):
            acc += res.results[b * 4 + hg]["out"]
        out[b] = (acc + bo.astype(np.float64)).astype(np.float32)
    return out

